# Initial kernel scaffold
#
"""CombinedRankingLoss Trainium2 Bass kernel.

Data-parallel over 8 NeuronCores: each core takes a [1024, 1024] slice of
scores/labels, computes partial sums of the three loss components, host
combines the 8 partial vectors into the final scalar.

Math (per row, x = scores/clip(temp), b = labels>0, e = exp(x)):
  ListMLE with labels in {0..4} needs, per element i with label v=l_i>=1:
    T_i = D_v - I_v(i) + e_i
  where I_v(i) = inclusive prefix sum of e*[l==v] along the row and
  D_v = sum of e over labels <= v.  In label-sorted order T is the plain
  suffix-sum, so  sum_masked ll = sum_masked x - sum_masked ln T  and
    per_list = (sum ln T - x_masked_sum) / (K + eps).
  The masked ln-sum is computed as Ln(1 + Q) with
    Q = sum_v (l==v)*G'_v + (l>0)*e,   G'_v = (D_v - 1) - I_v
  so unmasked elements contribute ln(1)=0 and ACT's accum_out gives the
  per-row sum for free.
  Focal/BCE via u = x*(1-2b):  ce = softplus(u) = -ln(sigmoid(-u)),
    (p-b)^2 = sigmoid(u)^2 = (1 - sigmoid(-u))^2.
"""

import numpy as np

import concourse.bass as bass
import concourse.bass_isa as bass_isa
import concourse.mybir as mybir
from concourse.tile import TileContext

AL = mybir.AluOpType
AF = mybir.ActivationFunctionType
F32 = mybir.dt.float32
BF16 = mybir.dt.bfloat16
I32 = mybir.dt.int32

N_CORES = 8
B_FULL = 8192
N = 1024
ROWS_PER_CORE = B_FULL // N_CORES  # 1024
EPS = 1e-10
LS_OVER_N = 0.1 / N


def build_nc(rows=ROWS_PER_CORE, n=N, groups_per_step=2):
    """Build the per-core Bass program. rows must be a multiple of
    128*groups_per_step."""
    P = 128
    G = rows // P                       # row-groups of 128
    S_STEPS = G // groups_per_step      # loop steps
    GS = groups_per_step
    W = GS * n                          # free elems per bulk op

    nc = bass.Bass()
    d_scores = nc.dram_tensor("scores", [rows, n], F32, kind="ExternalInput")
    d_labels = nc.dram_tensor("labels", [rows, n], I32, kind="ExternalInput")
    d_temp = nc.dram_tensor("temperature", [1], F32, kind="ExternalInput")
    d_out = nc.dram_tensor("out", [1, 8], F32, kind="ExternalOutput")

    sc_re = d_scores.rearrange("(g p) n -> p g n", p=P)
    lb_re = d_labels.rearrange("(g p) n -> p g n", p=P)
    tp_re = d_temp.rearrange("(p a) -> p a", p=1)

    with TileContext(nc) as tc:
        with (
            tc.tile_pool(name="const", bufs=1) as cpool,
            tc.tile_pool(name="io", bufs=2) as iopool,
            tc.tile_pool(name="wk", bufs=1) as wk,
            tc.tile_pool(name="stat", bufs=1) as st,
        ):
            # ---- constants / stats ----
            ones_b = cpool.tile([P, n], BF16, tag="ones")
            nc.vector.memset(ones_b[:], 1.0)
            t_raw = cpool.tile([1, 1], F32, tag="traw")
            t_clip = cpool.tile([1, 1], F32, tag="tclip")
            t_inv1 = cpool.tile([1, 1], F32, tag="tinv1")
            inv_t = cpool.tile([P, 1], F32, tag="invt")
            nc.sync.dma_start(t_raw[:], tp_re[:, :])
            nc.vector.tensor_scalar(t_clip[:], t_raw[:], 0.1, 5.0, AL.max, AL.min)
            nc.vector.reciprocal(t_inv1[:], t_clip[:])
            nc.gpsimd.partition_broadcast(inv_t[:], t_inv1[:])
            neg_inv_t = cpool.tile([P, 1], F32, tag="ninvt")
            nc.vector.tensor_scalar(neg_inv_t[:], inv_t[:], -1.0, None, AL.mult)

            # per-row stats, one column per row-group
            tS = st.tile([P, G], F32, tag="S")
            tE = [st.tile([P, G], F32, tag=f"E{v}") for v in range(1, 5)]
            tK = st.tile([P, G], F32, tag="K")
            tL = st.tile([P, G], F32, tag="L")
            tWx = st.tile([P, G], F32, tag="Wx")
            tU = st.tile([P, G], F32, tag="U")
            tLnp = st.tile([P, G], F32, tag="Lnp")
            tF = st.tile([P, G], F32, tag="F")

            for s in range(S_STEPS):
                g0 = s * GS
                sl = slice(g0, g0 + GS)
                sco = iopool.tile([P, GS, n], F32, tag="sco")
                lab = iopool.tile([P, GS, n], I32, tag="lab")
                nc.sync.dma_start(sco[:], sc_re[:, sl, :])
                nc.sync.dma_start(lab[:], lb_re[:, sl, :])
                scof = sco[:].rearrange("p g n -> p (g n)")
                labf = lab[:].rearrange("p g n -> p (g n)")

                e = wk.tile([P, GS, n], BF16, tag="e")
                ef = e[:].rearrange("p g n -> p (g n)")
                me = [wk.tile([P, GS, n], BF16, tag=f"me{v}") for v in range(1, 5)]
                gp = [wk.tile([P, GS, n], BF16, tag=f"gp{v}") for v in range(1, 5)]
                uv = [wk.tile([P, GS, n], BF16, tag=f"uv{v}") for v in range(1, 5)]
                mep = wk.tile([P, GS, n], BF16, tag="mep")
                dp = wk.tile([P, 4, GS], F32, tag="dp")  # D'_v per group
                q12 = wk.tile([P, GS, n], BF16, tag="q12")
                q34 = wk.tile([P, GS, n], BF16, tag="q34")
                qq = wk.tile([P, GS, n], BF16, tag="qq")
                mpos = wk.tile([P, GS, n], BF16, tag="mpos")
                w_t = wk.tile([P, GS, n], F32, tag="w")
                ur = wk.tile([P, GS, n], F32, tag="ur")
                pneg = wk.tile([P, GS, n], BF16, tag="pneg")
                lnp = wk.tile([P, GS, n], BF16, tag="lnp")
                s1 = wk.tile([P, GS, n], BF16, tag="s1")
                s2 = wk.tile([P, GS, n], BF16, tag="s2")
                scr = wk.tile([P, GS, n], BF16, tag="scr")

                # e = exp(x), accum S per row-group
                for gi in range(GS):
                    nc.scalar.activation(
                        e[:, gi, :], sco[:, gi, :], AF.Exp,
                        scale=inv_t[:], accum_out=tS[:, g0 + gi : g0 + gi + 1],
                    )

                # masked e per label (GPSIMD), accum E_v
                for v in range(1, 5):
                    nc.gpsimd.scalar_tensor_tensor(
                        me[v - 1][:].rearrange("p g n -> p (g n)"),
                        labf, float(v), ef, AL.is_equal, AL.mult,
                        accum_out=None,
                    )
                    # per-group row sums E_v (needed for D'_v)
                    for gi in range(GS):
                        nc.vector.tensor_reduce(
                            tE[v - 1][:, g0 + gi : g0 + gi + 1],
                            me[v - 1][:, gi, :],
                            mybir.AxisListType.X, AL.add,
                        )

                # D'_4 = S-1 ; D'_v = D'_{v+1} - E_{v+1}
                nc.vector.tensor_scalar(
                    dp[:, 3, :], tS[:, sl], -1.0, None, AL.add)
                for v in (3, 2, 1):
                    nc.vector.tensor_tensor(
                        dp[:, v - 1, :], dp[:, v, :], tE[v][:, sl], AL.subtract)

                # scans: G'_v = D'_v - inclusive_prefix(me_v)
                for v in range(1, 5):
                    eng = nc.gpsimd if v <= 2 else nc.vector
                    for gi in range(GS):
                        eng.tensor_tensor_scan(
                            gp[v - 1][:, gi, :], ones_b[:], me[v - 1][:, gi, :],
                            dp[:, v - 1, g0 + gi : g0 + gi + 1]
                            if False else dp[:, v - 1, gi : gi + 1],
                            AL.mult, AL.subtract,
                        )

                # u_v = (l==v) * G'_v   (GPSIMD)
                for v in range(1, 5):
                    nc.gpsimd.scalar_tensor_tensor(
                        uv[v - 1][:].rearrange("p g n -> p (g n)"),
                        labf, float(v),
                        gp[v - 1][:].rearrange("p g n -> p (g n)"),
                        AL.is_equal, AL.mult,
                    )
                # me_pos = (l>=1) * e (GPSIMD)
                nc.gpsimd.scalar_tensor_tensor(
                    mep[:].rearrange("p g n -> p (g n)"), labf, 1.0, ef,
                    AL.is_ge, AL.mult,
                )
                # Q = u1+u2+u3+u4+me_pos (DVE bf16 2x)
                fl = lambda t: t[:].rearrange("p g n -> p (g n)")
                nc.vector.tensor_tensor(fl(q12), fl(uv[0]), fl(uv[1]), AL.add)
                nc.vector.tensor_tensor(fl(q34), fl(uv[2]), fl(uv[3]), AL.add)
                nc.vector.tensor_tensor(fl(qq), fl(q12), fl(q34), AL.add)
                nc.vector.tensor_tensor(fl(qq), fl(qq), fl(mep), AL.add)
                # L = sum ln(1+Q) per row
                for gi in range(GS):
                    nc.scalar.activation(
                        qq[:, gi, :], qq[:, gi, :], AF.Ln, bias=1.0,
                        accum_out=tL[:, g0 + gi : g0 + gi + 1],
                    )

                # K = sum(l>=1) per row (DVE ts 2x, accum)
                for gi in range(GS):
                    nc.vector.tensor_scalar(
                        mpos[:, gi, :], lab[:, gi, :], 1, None, AL.is_ge,
                        accum_out=tK[:, g0 + gi : g0 + gi + 1],
                    )
                # w = (l>=1)*scores, accum Wx  (GPSIMD f32)
                for gi in range(GS):
                    nc.gpsimd.scalar_tensor_tensor(
                        w_t[:, gi, :], lab[:, gi, :], 1, sco[:, gi, :],
                        AL.is_ge, AL.mult,
                        accum_out=tWx[:, g0 + gi : g0 + gi + 1],
                    )
                # u_raw = scores - 2w, accum U  (GPSIMD f32)
                for gi in range(GS):
                    nc.gpsimd.scalar_tensor_tensor(
                        ur[:, gi, :], w_t[:, gi, :], -2.0, sco[:, gi, :],
                        AL.mult, AL.add,
                        accum_out=tU[:, g0 + gi : g0 + gi + 1],
                    )
                # p_neg = sigmoid(-x*(1-2b))
                nc.scalar.activation(fl(pneg), fl(ur), AF.Sigmoid,
                                     scale=neg_inv_t[:])
                # lnp = ln(p_neg), accum per row
                for gi in range(GS):
                    nc.scalar.activation(
                        lnp[:, gi, :], pneg[:, gi, :], AF.Ln,
                        accum_out=tLnp[:, g0 + gi : g0 + gi + 1],
                    )
                # s1 = 1-p_neg ; s2 = s1^2 ; F = sum s2*lnp
                nc.vector.tensor_scalar(fl(s1), fl(pneg), -1.0, 1.0,
                                        AL.mult, AL.add)
                nc.vector.tensor_tensor(fl(s2), fl(s1), fl(s1), AL.mult)
                for gi in range(GS):
                    nc.vector.tensor_tensor_reduce(
                        scr[:, gi, :], s2[:, gi, :], lnp[:, gi, :],
                        1.0, 0.0, AL.mult, AL.add,
                        accum_out=tF[:, g0 + gi : g0 + gi + 1],
                    )

            # ---- epilogue: per-row listmle, then partial sums ----
            ep = st.tile([P, G], F32, tag="ep1")
            ep2 = st.tile([P, G], F32, tag="ep2")
            stat = st.tile([P, 8], F32, tag="stat")
            statr = st.tile([P, 8], F32, tag="statr")
            nc.vector.memset(stat[:], 0.0)
            # ep = (L - inv_t*Wx)
            nc.vector.tensor_scalar(ep[:], tWx[:], inv_t[:], None, AL.mult)
            nc.vector.tensor_tensor(ep[:], tL[:], ep[:], AL.subtract)
            # ep2 = 1/(K+eps)
            nc.vector.tensor_scalar(ep2[:], tK[:], EPS, None, AL.add)
            nc.vector.reciprocal(ep2[:], ep2[:])
            nc.vector.tensor_tensor(ep[:], ep[:], ep2[:], AL.mult)
            nc.vector.tensor_reduce(stat[:, 0:1], ep[:], mybir.AxisListType.X, AL.add)
            nc.vector.tensor_reduce(stat[:, 1:2], tF[:], mybir.AxisListType.X, AL.add)
            nc.vector.tensor_reduce(stat[:, 2:3], tLnp[:], mybir.AxisListType.X, AL.add)
            nc.vector.tensor_reduce(stat[:, 3:4], tWx[:], mybir.AxisListType.X, AL.add)
            nc.vector.tensor_reduce(stat[:, 4:5], tU[:], mybir.AxisListType.X, AL.add)
            nc.gpsimd.partition_all_reduce(statr[:], stat[:], 128,
                                           bass_isa.ReduceOp.add)
            nc.sync.dma_start(d_out[:, :], statr[0:1, :])
    return nc


def combine_partials(parts, temp_val, b_full, n):
    """parts: [n_cores, 8] f32 partial sums. Host combine (scalars only)."""
    parts = np.asarray(parts, dtype=np.float64)
    inv_t = 1.0 / float(np.clip(temp_val, 0.1, 5.0))
    A = parts[:, 0].sum()      # sum of per-row listmle
    F_ = parts[:, 1].sum()     # sum s2*lnp
    C = parts[:, 2].sum()      # sum lnp
    Wx = parts[:, 3].sum()     # sum b*scores
    U = parts[:, 4].sum()      # sum scores*(1-2b)
    cnt = float(b_full) * n
    listmle = A / b_full
    ce_sum = -C
    focal = (0.25 * -F_) / cnt
    sum_x = inv_t * (U + 2.0 * Wx)
    sum_xb = inv_t * Wx
    smooth = (ce_sum + 0.1 * sum_xb - (0.1 / n) * sum_x) / cnt
    total = 0.7 * listmle + 0.3 * focal + 0.1 * smooth
    return np.float32(total)


_CACHED = {}


def kernel(scores, temperature, labels):
    from concourse.bass_utils import run_bass_kernel_spmd

    scores = np.ascontiguousarray(np.asarray(scores), dtype=np.float32)
    labels = np.ascontiguousarray(np.asarray(labels), dtype=np.int32)
    temperature = np.asarray(temperature, dtype=np.float32).reshape(1)

    key = (scores.shape, labels.shape)
    if key not in _CACHED:
        _CACHED[key] = build_nc(rows=scores.shape[0] // N_CORES, n=scores.shape[1])
    nc = _CACHED[key]

    rows = scores.shape[0] // N_CORES
    in_maps = [
        {
            "scores": scores[i * rows : (i + 1) * rows],
            "labels": labels[i * rows : (i + 1) * rows],
            "temperature": temperature,
        }
        for i in range(N_CORES)
    ]
    res = run_bass_kernel_spmd(nc, in_maps, list(range(N_CORES)))
    parts = np.stack([res.results[i]["out"][0] for i in range(N_CORES)])
    return combine_partials(parts, temperature[0], scores.shape[0],
                            scores.shape[1])


# revision 13
# speedup vs baseline: 1.0182x; 1.0182x over previous
"""CombinedRankingLoss Trainium2 Bass kernel.

Data-parallel over 8 NeuronCores: each core takes a [1024, 1024] slice of
scores/labels, computes partial sums of the three loss components, host
combines the 8 partial vectors into the final scalar.

Math (per row, x = scores/clip(temp), b = labels>0, e = exp(x)):
  ListMLE with labels in {0..4} needs, per element i with label v=l_i>=1:
    T_i = D_v - I_v(i) + e_i
  where I_v(i) = inclusive prefix sum of e*[l==v] along the row and
  D_v = sum of e over labels <= v.  In label-sorted order T is the plain
  suffix-sum, so  sum_masked ll = sum_masked x - sum_masked ln T  and
    per_list = (sum ln T - x_masked_sum) / (K + eps).
  The masked ln-sum is computed as Ln(1 + Q) with
    Q = sum_v (l==v)*G'_v + (l>0)*e,   G'_v = (D_v - 1) - I_v
  so unmasked elements contribute ln(1)=0 and ACT's accum_out gives the
  per-row sum for free.
  Focal/BCE via u = x*(1-2b):  ce = softplus(u) = -ln(sigmoid(-u)),
    (p-b)^2 = sigmoid(u)^2 = (1 - sigmoid(-u))^2.
"""

import numpy as np

import concourse.bass as bass
import concourse.bacc as bacc
import concourse.bass_isa as bass_isa
import concourse.mybir as mybir
from concourse.tile import TileContext

AL = mybir.AluOpType
AF = mybir.ActivationFunctionType
F32 = mybir.dt.float32
BF16 = mybir.dt.bfloat16
I32 = mybir.dt.int32

N_CORES = 8
B_FULL = 8192
N = 1024
ROWS_PER_CORE = B_FULL // N_CORES  # 1024
EPS = 1e-10
LS_OVER_N = 0.1 / N


def build_nc(rows=ROWS_PER_CORE, n=N, groups_per_step=2, stages=5, time_reps=1):
    """Build the per-core Bass program. rows must be a multiple of
    128*groups_per_step."""
    P = 128
    G = rows // P                       # row-groups of 128
    S_STEPS = G // groups_per_step      # loop steps
    GS = groups_per_step
    W = GS * n                          # free elems per bulk op

    nc = bacc.Bacc("TRN2", target_bir_lowering=False, debug=False)
    d_scores = nc.dram_tensor("scores", [rows, n], F32, kind="ExternalInput")
    d_labels = nc.dram_tensor("labels", [rows, n], I32, kind="ExternalInput")
    d_temp = nc.dram_tensor("temperature", [1], F32, kind="ExternalInput")
    d_out = nc.dram_tensor("out", [1, 8], F32, kind="ExternalOutput")

    sc_re = d_scores.rearrange("(g p) n -> p g n", p=P)
    lb_re = d_labels.rearrange("(g p) n -> p g n", p=P)
    tp_re = d_temp.rearrange("(p a) -> p a", p=1)

    with TileContext(nc) as tc:
        with (
            tc.tile_pool(name="const", bufs=1) as cpool,
            tc.tile_pool(name="io", bufs=2) as iopool,
            tc.tile_pool(name="wk", bufs=1) as wk,
            tc.tile_pool(name="stat", bufs=1) as st,
            tc.tile_pool(name="ps", bufs=1, space="PSUM") as pspool,
        ):
            # ---- constants / stats ----
            ones_b = cpool.tile([P, n], BF16, tag="ones", name="ones")
            nc.vector.memset(ones_b[:], 1.0)
            t_raw = cpool.tile([P, 1], F32, tag="traw", name="traw")
            t_clip = cpool.tile([P, 1], F32, tag="tclip", name="tclip")
            inv_t = cpool.tile([P, 1], F32, tag="invt", name="invt")
            nc.sync.dma_start(t_raw[:], tp_re[:, :].partition_broadcast(P))
            nc.vector.tensor_scalar(t_clip[:], t_raw[:], 0.1, 5.0, AL.max, AL.min)
            nc.vector.reciprocal(inv_t[:], t_clip[:])
            neg_inv_t = cpool.tile([P, 1], F32, tag="ninvt", name="ninvt")
            nc.vector.tensor_scalar(neg_inv_t[:], inv_t[:], -1.0, None, AL.mult)

            # per-row stats, one column per row-group
            tS = st.tile([P, G], F32, tag="S", name="S")
            tK = st.tile([P, G], F32, tag="K", name="K")
            _stats_to_clear = []
            tL = st.tile([P, G], F32, tag="L", name="L")
            tWx = st.tile([P, G], F32, tag="Wx", name="Wx")
            tU = st.tile([P, G], F32, tag="U", name="U")
            tLnp = st.tile([P, G], F32, tag="Lnp", name="Lnp")
            tF = st.tile([P, G], F32, tag="F", name="F")
            for _t in (tS, tK, tL, tWx, tU, tLnp, tF):
                nc.vector.memset(_t[:], 0.0)

            from contextlib import nullcontext
            loop_cm = tc.For_i(0, time_reps, 1) if time_reps > 1 else nullcontext()
            with loop_cm:
              for s in range(S_STEPS):
                g0 = s * GS
                sl = slice(g0, g0 + GS)
                sco = iopool.tile([P, GS, n], F32, tag="sco", name="sco")
                lab = iopool.tile([P, GS, n], I32, tag="lab", name="lab")
                nc.sync.dma_start(sco[:], sc_re[:, sl, :])
                nc.sync.dma_start(lab[:], lb_re[:, sl, :])
                scof = sco[:].rearrange("p g n -> p (g n)")
                labf = lab[:].rearrange("p g n -> p (g n)")

                e = wk.tile([P, GS, n], BF16, tag="e", name="e")
                lab_bf = wk.tile([P, GS, n], BF16, tag="labbf", name="labbf")
                m = [wk.tile([P, GS, n], BF16, tag=f"m{v}", name=f"m{v}") for v in range(1, 5)]
                me = [wk.tile([P, GS, n], BF16, tag=f"me{v}", name=f"me{v}") for v in range(1, 5)]
                buf = [wk.tile([P, GS, n + 1], BF16, tag=f"buf{v}", name=f"buf{v}") for v in range(1, 5)]
                uv = [wk.tile([P, GS, n], BF16, tag=f"uv{v}", name=f"uv{v}") for v in range(1, 5)]
                dp = wk.tile([P, 4, GS], F32, tag="dp", name="dp")
                q12 = wk.tile([P, GS, n], BF16, tag="q12", name="q12")
                q34 = wk.tile([P, GS, n], BF16, tag="q34", name="q34")
                qq = wk.tile([P, GS, n], BF16, tag="qq", name="qq")
                mpos = wk.tile([P, GS, n], BF16, tag="mpos", name="mpos")
                w_t = wk.tile([P, GS, n], F32, tag="w", name="w")
                ur = wk.tile([P, GS, n], F32, tag="ur", name="ur")
                pneg = wk.tile([P, GS, n], BF16, tag="pneg", name="pneg")
                lnp = wk.tile([P, GS, n], BF16, tag="lnp", name="lnp")
                s1 = wk.tile([P, GS, n], BF16, tag="s1", name="s1")
                s2 = wk.tile([P, GS, n], BF16, tag="s2", name="s2")
                gg = wk.tile([P, GS, n], BF16, tag="gg", name="gg")
                fl = lambda t: t[:].rearrange("p g n -> p (g n)")

                # e = exp(x), accum S per row-group   (ACT)
                for gi in range(GS):
                    nc.scalar.activation(
                        e[:, gi, :], sco[:, gi, :], AF.Exp,
                        scale=inv_t[:], accum_out=tS[:, g0 + gi : g0 + gi + 1],
                    )

                if stages < 2:
                    continue
                # masks: m1,m2 on DVE from i32; m3,m4 on GP from lab_bf
                nc.gpsimd.tensor_copy(fl(lab_bf), labf)
                for v in (1, 2):
                    nc.vector.tensor_scalar(fl(m[v - 1]), labf, float(v), None,
                                            AL.is_equal)
                for v in (3, 4):
                    nc.gpsimd.tensor_scalar(fl(m[v - 1]), fl(lab_bf), float(v),
                                            None, AL.is_equal)
                # K = sum(l>0) per row (STT accum; elementwise out is m_pos)
                for gi in range(GS):
                    nc.vector.scalar_tensor_tensor(
                        mpos[:, gi, :], lab[:, gi, :], 1, ones_b[:],
                        AL.is_ge, AL.mult,
                        accum_out=tK[:, g0 + gi : g0 + gi + 1],
                    )

                if stages < 3:
                    continue
                # masked e (GPSIMD tt)
                for v in range(1, 5):
                    nc.gpsimd.tensor_tensor(fl(me[v - 1]), fl(m[v - 1]), fl(e),
                                            AL.mult)

                # D'_4 = S-1 ; D'_v = D'_{v+1} - E_{v+1}; E_v = -buf_v[:, :, n]
                # (scan tails used below, so dp for v<4 depends on scans of v+1)
                nc.vector.tensor_scalar(dp[:, 3, :], tS[:, sl], -1.0, None, AL.add)
                for v in (4, 3, 2, 1):
                    for gi in range(GS):
                        # inject D' into the scan: me[0] -= D'  => scan out
                        # = D' - inclusive_prefix(me)
                        nc.vector.scalar_tensor_tensor(
                            me[v - 1][:, gi, 0:1], dp[:, v - 1, gi : gi + 1],
                            -1.0, me[v - 1][:, gi, 0:1], AL.mult, AL.add,
                        )
                        # buf col0 = D' (exclusive value for j=0)
                        nc.vector.tensor_copy(buf[v - 1][:, gi, 0:1],
                                              dp[:, v - 1, gi : gi + 1])
                        nc.vector.tensor_tensor_scan(
                            buf[v - 1][:, gi, 1 : n + 1], ones_b[:],
                            me[v - 1][:, gi, :], 0.0, AL.mult, AL.subtract,
                        )
                    if v > 1:
                        # D'_{v-1} = D'_v - E_v = D'_v + (buf_v[n] - D'_v) ...
                        # buf_v[:, :, n] = D'_v - sum(me_v) = D'_v - E_v + D'corr
                        nc.vector.tensor_copy(dp[:, v - 2, :], buf[v - 1][:, :, n])
                        nc.vector.tensor_scalar(dp[:, v - 2, :], dp[:, v - 2, :],
                                                -1.0, None, AL.add)

                # u_v = m_v * buf_v[:, :, 0:n] ; Q = sum (GPSIMD)
                for v in range(1, 5):
                    nc.gpsimd.tensor_tensor(uv[v - 1][:], m[v - 1][:],
                                            buf[v - 1][:, :, 0:n], AL.mult)
                if stages < 4:
                    continue
                nc.gpsimd.tensor_tensor(fl(q12), fl(uv[0]), fl(uv[1]), AL.add)
                nc.gpsimd.tensor_tensor(fl(q34), fl(uv[2]), fl(uv[3]), AL.add)
                nc.vector.tensor_tensor(fl(qq), fl(q12), fl(q34), AL.add)

                # L = sum ln(1+Q) per row   (ACT)
                for gi in range(GS):
                    nc.scalar.activation(
                        qq[:, gi, :], qq[:, gi, :], AF.Ln, bias=1.0,
                        accum_out=tL[:, g0 + gi : g0 + gi + 1],
                    )

                if stages < 5:
                    continue
                # w = m_pos*scores (+Wx), u_raw = scores - 2w (+U)
                for gi in range(GS):
                    nc.vector.scalar_tensor_tensor(
                        w_t[:, gi, :], lab[:, gi, :], 1, sco[:, gi, :],
                        AL.is_ge, AL.mult,
                        accum_out=tWx[:, g0 + gi : g0 + gi + 1],
                    )
                for gi in range(GS):
                    nc.vector.scalar_tensor_tensor(
                        ur[:, gi, :], w_t[:, gi, :], -2.0, sco[:, gi, :],
                        AL.mult, AL.add,
                        accum_out=tU[:, g0 + gi : g0 + gi + 1],
                    )
                # p_neg = sigmoid(-x*(1-2b)); lnp = ln(p_neg) (+sum)
                nc.scalar.activation(fl(pneg), fl(ur), AF.Sigmoid,
                                     scale=neg_inv_t[:])
                for gi in range(GS):
                    nc.scalar.activation(
                        lnp[:, gi, :], pneg[:, gi, :], AF.Ln,
                        accum_out=tLnp[:, g0 + gi : g0 + gi + 1],
                    )
                # s1 = 1-p_neg ; s2 = s1^2 ; g = s2*lnp ; F = rowsum(g)
                nc.vector.tensor_scalar(fl(s1), fl(pneg), -1.0, 1.0,
                                        AL.mult, AL.add)
                nc.gpsimd.tensor_tensor(fl(s2), fl(s1), fl(s1), AL.mult)
                nc.gpsimd.tensor_tensor(fl(gg), fl(s2), fl(lnp), AL.mult)
                nc.vector.tensor_reduce(tF[:, sl], gg[:],
                                        mybir.AxisListType.X, AL.add)

            # ---- epilogue: per-row listmle, then partial sums ----
            ep = st.tile([P, G], F32, tag="ep1", name="ep1")
            ep2 = st.tile([P, G], F32, tag="ep2", name="ep2")
            stat = st.tile([P, 8], F32, tag="stat", name="stat")
            statr = st.tile([P, 8], F32, tag="statr", name="statr")
            nc.vector.memset(stat[:], 0.0)
            # ep = (L - inv_t*Wx)
            nc.vector.tensor_scalar(ep[:], tWx[:], inv_t[:], None, AL.mult)
            nc.vector.tensor_tensor(ep[:], tL[:], ep[:], AL.subtract)
            # ep2 = 1/(K+eps)
            nc.vector.tensor_scalar(ep2[:], tK[:], EPS, None, AL.add)
            nc.vector.reciprocal(ep2[:], ep2[:])
            nc.vector.tensor_tensor(ep[:], ep[:], ep2[:], AL.mult)
            nc.vector.tensor_reduce(stat[:, 0:1], ep[:], mybir.AxisListType.X, AL.add)
            nc.vector.tensor_reduce(stat[:, 1:2], tF[:], mybir.AxisListType.X, AL.add)
            nc.vector.tensor_reduce(stat[:, 2:3], tLnp[:], mybir.AxisListType.X, AL.add)
            nc.vector.tensor_reduce(stat[:, 3:4], tWx[:], mybir.AxisListType.X, AL.add)
            nc.vector.tensor_reduce(stat[:, 4:5], tU[:], mybir.AxisListType.X, AL.add)
            ones_f = cpool.tile([P, 1], F32, tag="onesf", name="onesf")
            nc.vector.memset(ones_f[:], 1.0)
            ps_out = pspool.tile([1, 8], F32, tag="psout", name="psout")
            nc.tensor.matmul(ps_out[:], ones_f[:], stat[:], start=True, stop=True)
            nc.vector.tensor_copy(statr[0:1, :], ps_out[:])
            nc.sync.dma_start(d_out[:, :], statr[0:1, :])
    nc.compile()
    return nc


def combine_partials(parts, temp_val, b_full, n):
    """parts: [n_cores, 8] f32 partial sums. Host combine (scalars only)."""
    parts = np.asarray(parts, dtype=np.float64)
    inv_t = 1.0 / float(np.clip(temp_val, 0.1, 5.0))
    A = parts[:, 0].sum()      # sum of per-row listmle
    F_ = parts[:, 1].sum()     # sum s2*lnp
    C = parts[:, 2].sum()      # sum lnp
    Wx = parts[:, 3].sum()     # sum b*scores
    U = parts[:, 4].sum()      # sum scores*(1-2b)
    cnt = float(b_full) * n
    listmle = A / b_full
    ce_sum = -C
    focal = (0.25 * -F_) / cnt
    sum_x = inv_t * (U + 2.0 * Wx)
    sum_xb = inv_t * Wx
    smooth = (ce_sum + 0.1 * sum_xb - (0.1 / n) * sum_x) / cnt
    total = 0.7 * listmle + 0.3 * focal + 0.1 * smooth
    return np.float32(total)


_CACHED = {}


def kernel(scores, temperature, labels):
    from concourse.bass_utils import run_bass_kernel_spmd

    scores = np.ascontiguousarray(np.asarray(scores), dtype=np.float32)
    labels = np.ascontiguousarray(np.asarray(labels), dtype=np.int32)
    temperature = np.asarray(temperature, dtype=np.float32).reshape(1)

    key = (scores.shape, labels.shape)
    if key not in _CACHED:
        _CACHED[key] = build_nc(rows=scores.shape[0] // N_CORES, n=scores.shape[1])
    nc = _CACHED[key]

    rows = scores.shape[0] // N_CORES
    in_maps = [
        {
            "scores": scores[i * rows : (i + 1) * rows],
            "labels": labels[i * rows : (i + 1) * rows],
            "temperature": temperature,
        }
        for i in range(N_CORES)
    ]
    res = run_bass_kernel_spmd(nc, in_maps, list(range(N_CORES)))
    parts = np.stack([res.results[i]["out"][0] for i in range(N_CORES)])
    return combine_partials(parts, temperature[0], scores.shape[0],
                            scores.shape[1])
